# revision 13
# baseline (speedup 1.0000x reference)
"""Trainium2 Bass kernel for masked two-template sparse attention.

Model (per sample, fp32 reference):
    qkv = (x @ W_qkv.T) * mask          mask: temp_mask on first 64 tokens, 1 elsewhere
    q,k,v split into 12 heads x 64
    template tokens (first 128) attend to template tokens only
    search tokens (last 324) attend to all 452 tokens
    out = concat(attn outputs) @ W_proj.T + b_proj

Sharding: data-parallel over batch, 32 samples -> 4 per NeuronCore x 8 cores.
All attention math in "transposed" layout (channels on partitions):
    x^T (PE transpose) -> q^T,k^T = Wqkv^T.T @ x^T ; v natural = x^T.T @ Wv^T
    S^T = k^T.T @ q^T  per key-chunk; the template block IS key-chunk 0 vs
      q 0:128, so chunk 0 streams all 452 queries and the other chunks only
      the 324 search queries. Head pair row-tiled at (0,0)/(64,0) into the
      two banks of one PSUM tile.
    E^T = exp(S^T * scale)             (no max subtraction; |S| <~ 6)
    [attn^T_unnorm ; sums] = [v_h|ones].T @ E^T  (one matmul per head/k-chunk;
                              partitions 0:64 = attn.V, 64:128 = denominator)
    attn^T = attn^T_unnorm * recip(sums)         (partition-shifted DVE ops)
    y = attn^T.T @ Wp^T                (+ bias added on host)

Scheduling: the engine queues are strict FIFO at runtime, so a stalled
attention matmul blocks every later (ready) instruction behind it. To keep
the PE fed, emission is software-pipelined: sample i's projection blocks
are interleaved 1:2 with sample i-1's attention blocks, loads are
prefetched one sample ahead (Pool engine), and PV/norm(p-1) is emitted
after scores(p) so each exp has a full block of slack before its consumer.
"""

import numpy as np
import ml_dtypes

import concourse.bass as bass
import concourse.mybir as mybir
import concourse.tile as tile
from concourse.bass_utils import run_bass_kernel_spmd
from concourse.masks import make_identity

# ---------------- configuration ----------------
PROJ_DT_NAME = "bfloat16"
ATT_DT_NAME = "bfloat16"
TRACE = False        # request NTFF profile on run
PHASES = 99          # kept for test.py compat (unused)
REPS = 1             # timing: repeat the whole computation inside the NEFF
# Timing-ablation switches (results become garbage; ONLY for HW attribution).
# Each entry shrinks the free dim of one instruction class to ~8 while keeping
# instruction count and dependency shape: {"scores","pv","qkproj","vproj",
# "outproj","exp","dve"}
ABLATE = set()

NCORES = 8
S = 4                # samples per core
N, C, H, DH = 452, 768, 12, 64
NMT, NS = 128, 324   # template tokens / search tokens
SCALE = DH ** -0.5
TCH = [(0, 128), (128, 256), (256, 384), (384, 452)]  # token/key chunks
KC = 6               # channel chunks of 128
NPAD = 512           # padded token width for x^T storage

_F32 = mybir.dt.float32


def _legalize_waits(nc, max_waits=1):
    """This container's walrus accepts at most one sync-wait per instruction;
    hoist extra waits onto dedicated NOPs in front of the instruction."""
    n_split = 0
    for f in nc.m.functions:
        for bb in f.blocks:
            new_insts = []
            for inst in bb.instructions:
                si = inst.sync_info
                if si is not None and si.on_wait and len(si.on_wait) > max_waits:
                    waits = list(si.on_wait)
                    for i, w in enumerate(waits[:-max_waits]):
                        new_insts.append(
                            mybir.InstNoOp(
                                name=f"{inst.name}-w{i}",
                                sync_info=mybir.SyncInfo(on_wait=[w], on_update=[]),
                                bass_nofuse=True,
                                engine=inst.engine,
                            )
                        )
                    si.on_wait = waits[-max_waits:]
                    n_split += 1
                new_insts.append(inst)
            bb.instructions = new_insts
    return n_split


def build_module():
    pdt = getattr(mybir.dt, PROJ_DT_NAME)
    adt = getattr(mybir.dt, ATT_DT_NAME)

    nc = bass.Bass("TRN2", target_bir_lowering=False, debug=False)
    x_d = nc.dram_tensor("x", [S, N, C], _F32, kind="ExternalInput").ap()
    m_d = nc.dram_tensor("tmask", [S, 64], _F32, kind="ExternalInput").ap()
    wq_d = nc.dram_tensor("wqkvT", [C, 3 * C], pdt, kind="ExternalInput").ap()
    wp_d = nc.dram_tensor("wpT", [C, C], pdt, kind="ExternalInput").ap()
    y_d = nc.dram_tensor("y", [S, N, C], _F32, kind="ExternalOutput").ap()

    Exp = mybir.ActivationFunctionType.Exp

    def sz(cls, full):
        return 8 if cls in ABLATE else full

    with tile.TileContext(nc) as tc:
        with (
            tc.tile_pool(name="const", bufs=1) as cp,
            tc.tile_pool(name="work", bufs=1) as wk,
            # PSUM: 8 banks total.
            #  pps: 2-bank f32 [128,1024] - one tile per score key-chunk,
            #       head 0 in bank0, head 1 in bank1.   2 bufs -> 4 banks
            #  pacc: 1-bank (2KB) tiles - x^T transposes (bf16) and the
            #       qk/v projections (f32).             2 bufs -> 2 banks
            #  ppv: 1-bank f32 [128,512] - PV accumulators and the output
            #       projection.                          2 bufs -> 2 banks
            tc.tile_pool(name="pps", bufs=2, space="PSUM") as pps,
            tc.tile_pool(name="pacc", bufs=2, space="PSUM") as pacc,
            tc.tile_pool(name="ppv", bufs=2, space="PSUM") as ppv,
        ):
            # ---- persistent constants ----
            wq_sb = []
            for i in range(KC):
                w = cp.tile([128, 3 * C], pdt, name=f"wq{i}", tag=f"wq{i}")
                nc.scalar.dma_start(w[:, :], wq_d[i * 128:(i + 1) * 128, :])
                wq_sb.append(w)
            wp_sb = []
            for i in range(KC):
                w = cp.tile([128, C], pdt, name=f"wp{i}", tag=f"wp{i}")
                nc.scalar.dma_start(w[:, :], wp_d[i * 128:(i + 1) * 128, :])
                wp_sb.append(w)
            ident = cp.tile([128, 128], adt, name="ident", tag="ident")
            make_identity(nc, ident)

            niter = REPS * S

            def emit_load(it):
                """DMA + mask + cast for iteration it (runs one sample ahead;
                Pool engine only, so it never blocks the PE pipeline)."""
                s = it % S
                st = {}
                xn = wk.tile([128, 4, C], _F32, name="xn", tag="xn", bufs=2)
                nc.gpsimd.memset(xn[64:128, 3, :], 0.0)
                nc.sync.dma_start(
                    xn[:, 0:3, :],
                    x_d[s, 0:384, :].rearrange("(c p) d -> p c d", p=128),
                )
                nc.sync.dma_start(xn[0:68, 3, :], x_d[s, 384:452, :])
                msk = wk.tile([64, 1], _F32, name="msk", tag="msk", bufs=2)
                nc.sync.dma_start(msk[:, :], m_d[s, :].unsqueeze(1))
                nc.gpsimd.tensor_scalar_mul(xn[0:64, 0, :], xn[0:64, 0, :], msk[0:64, :])
                xnc = wk.tile([128, 4, C], adt, name="xnc", tag="xnc", bufs=2)
                nc.gpsimd.tensor_copy(xnc[:, :, :], xn[:, :, :])
                st["xnc"] = xnc
                return st

            def emit_proj(it, st):
                """x^T transposes + qk/v projections for iteration it.
                Yields after each block (~1us of dense PE work)."""
                xnc = st["xnc"]
                xTb = wk.tile([128, KC, NPAD], pdt, name="xTb", tag="xTb", bufs=2)
                st["xTb"] = xTb
                for ti in range(4):
                    ptr = pacc.tile([128, 1024], adt, name="ptr", tag="acc")
                    for cc in range(KC):
                        nc.tensor.transpose(
                            ptr[:, cc * 128:(cc + 1) * 128],
                            xnc[:, ti, cc * 128:(cc + 1) * 128],
                            ident[:, :],
                        )
                    if "dve" in ABLATE:
                        nc.scalar.copy(
                            xTb[:, 0:1, ti * 128:ti * 128 + 8],
                            ptr.rearrange("p (c k) -> p c k", k=128)[:, 0:1, 0:8],
                        )
                    else:
                        nc.scalar.copy(
                            xTb[:, :, ti * 128:(ti + 1) * 128],
                            ptr.rearrange("p (c k) -> p c k", k=128)[:, 0:KC, :],
                        )
                    yield
                xT = [xTb[:, cc, 0:N] for cc in range(KC)]

                qkT = []
                st["qkT"] = qkT
                QN = sz("qkproj", N)
                DN = sz("dve", N)
                for oc in range(12):
                    pq = pacc.tile([128, 512], _F32, name="pq", tag="acc")
                    for kc in range(KC):
                        nc.tensor.matmul(
                            pq[:, 0:QN],
                            wq_sb[kc][:, oc * 128:(oc + 1) * 128],
                            xTb[:, kc, 0:QN],
                            start=(kc == 0),
                            stop=(kc == KC - 1),
                        )
                    t = wk.tile([128, N], adt, name=f"qkT{oc}", tag=f"qkT{oc}", bufs=2)
                    nc.vector.tensor_copy(t[:, 0:DN], pq[:, 0:DN])
                    qkT.append(t)
                    yield

                vt = []
                st["vt"] = vt
                VN1 = sz("vproj", 512)
                VN2 = sz("vproj", 256)
                for ti, (t0, t1) in enumerate(TCH):
                    tsz = t1 - t0
                    t = wk.tile([128, H * 128], adt, name=f"v{ti}", tag=f"v{ti}", bufs=2)
                    th = t.rearrange("p (h c) -> p h c", c=128)
                    if it < 2:
                        # ones halves persist in the rotating buffers; only
                        # the first pass over each buffer needs the memset
                        nc.gpsimd.memset(th[:, :, DH:128], 1.0)
                    # pva/pvb de-interleaved: each accumulation group runs
                    # back-to-back on one PSUM bank, and the pool ping-pong
                    # gives the copy of one group slack behind the other's
                    # matmuls
                    pva = pacc.tile([128, 512], _F32, name="pva", tag="acc")
                    for kc in range(KC):
                        nc.tensor.matmul(
                            pva[0:tsz, 0:VN1],
                            xTb[:, kc, t0:t1],
                            wq_sb[kc][:, 1536:1536 + VN1],
                            start=(kc == 0),
                            stop=(kc == KC - 1),
                        )
                    if "dve" in ABLATE:
                        nc.scalar.copy(
                            th[0:tsz, 0:1, 0:8],
                            pva[0:tsz, :].rearrange("p (h c) -> p h c", c=DH)[:, 0:1, 0:8],
                        )
                    else:
                        nc.scalar.copy(
                            th[0:tsz, 0:8, 0:DH],
                            pva[0:tsz, :].rearrange("p (h c) -> p h c", c=DH),
                        )
                    yield
                    pvb = pacc.tile([128, 512], _F32, name="pvb", tag="acc")
                    for kc in range(KC):
                        nc.tensor.matmul(
                            pvb[0:tsz, 0:VN2],
                            xTb[:, kc, t0:t1],
                            wq_sb[kc][:, 2048:2048 + VN2],
                            start=(kc == 0),
                            stop=(kc == KC - 1),
                        )
                    if "dve" in ABLATE:
                        nc.scalar.copy(
                            th[0:tsz, 8:9, 0:8],
                            pvb[0:tsz, 0:256].rearrange("p (h c) -> p h c", c=DH)[:, 0:1, 0:8],
                        )
                    else:
                        nc.scalar.copy(
                            th[0:tsz, 8:12, 0:DH],
                            pvb[0:tsz, 0:256].rearrange("p (h c) -> p h c", c=DH),
                        )
                    vt.append(t)
                    yield

            def emit_attn(it, st):
                """Attention + output projection for iteration it. Yields
                after each small block; scores(p) are emitted before
                PV/norm(p-1) so every exp has slack before its consumer."""
                s = it % S
                qkT, vt = st["qkT"], st["vt"]
                es_all = [None] * 6
                st["attnT"] = [None] * 6

                def scores(p):
                    qc, kt = qkT[p], qkT[6 + p]
                    es = [None] * 4
                    es_all[p] = es
                    for kcj in (1, 2, 3, 0):
                        k0, k1 = TCH[kcj]
                        ksz = k1 - k0
                        qlo = 0 if kcj == 0 else NMT
                        qn = N - qlo
                        SQ = sz("scores", qn)
                        EQ = sz("exp", qn)
                        ps = pps.tile([128, 1024], _F32, name="ps", tag="ps")
                        for hh in range(2):
                            b0 = hh * 64
                            nc.tensor.matmul(
                                ps[0:ksz, hh * 512:hh * 512 + SQ],
                                kt[b0:b0 + 64, k0:k1],
                                qc[b0:b0 + 64, qlo:qlo + SQ],
                                start=True, stop=True,
                                tile_position=(b0, 0),
                                skip_group_check=True,
                            )
                        e = wk.tile(
                            [128, 2, qn], adt, name="es", tag=f"es{kcj}", bufs=3
                        )
                        nc.scalar.activation(
                            e[:, :, 0:EQ],
                            ps.rearrange("p (b k) -> p b k", k=512)[:, :, 0:EQ],
                            Exp,
                            scale=SCALE,
                        )
                        es[kcj] = e
                        yield

                def pv_norm(p):
                    es = es_all[p]
                    at = wk.tile(
                        [128, N], pdt, name=f"attnT{p}", tag=f"attnT{p}", bufs=2
                    )
                    st["attnT"][p] = at
                    for hh in range(2):
                        h = 2 * p + hh
                        pvps = ppv.tile([128, 512], _F32, name="pvps", tag="pv")
                        # chunk 0 (template+search, the longest exp) goes
                        # LAST: start=True on chunk 1 clears the bank, chunk
                        # 0 then overwrites cols 0:128 where has_written is
                        # still clear and accumulates on 128:452
                        for j, kcj in enumerate((1, 2, 3, 0)):
                            k0, k1 = TCH[kcj]
                            ksz = k1 - k0
                            qlo = 0 if kcj == 0 else NMT
                            PQ = sz("pv", N - qlo)
                            nc.tensor.matmul(
                                pvps[:, qlo:qlo + PQ],
                                vt[kcj][0:ksz, h * 128:(h + 1) * 128],
                                es[kcj][0:ksz, hh, 0:PQ],
                                start=(j == 0), stop=(j == 3),
                                skip_group_check=True,
                            )
                        DN = sz("dve", N)
                        r = wk.tile([64, N], _F32, name="r", tag="r", bufs=3)
                        nc.vector.reciprocal(r[:, 0:DN], pvps[64:128, 0:DN])
                        nc.vector.tensor_mul(
                            at[hh * 64:(hh + 1) * 64, 0:DN], pvps[0:64, 0:DN],
                            r[:, 0:DN]
                        )
                        yield

                # ready PV(p-1) blocks go BETWEEN the score blocks of
                # pair p, ahead of the ladder's FIFO stall points
                prev = None
                for p in range(6):
                    sg = scores(p)
                    vg = pv_norm(prev) if prev is not None else None
                    for g in (sg, sg, vg, sg, vg, sg):
                        if g is not None and next(g, StopIteration) is not StopIteration:
                            yield
                    prev = p
                yield from pv_norm(5)

                attnT = st["attnT"]
                ON1 = sz("outproj", 512)
                ON2 = sz("outproj", 256)
                YC = sz("exp", 768)
                for (q0, q1) in TCH:
                    qsz = q1 - q0
                    py = pps.tile([128, 1024], _F32, name="py", tag="ps")
                    for mc in range(KC):
                        nc.tensor.matmul(
                            py[0:qsz, 0:ON1],
                            attnT[mc][:, q0:q1],
                            wp_sb[mc][:, 0:ON1],
                            start=(mc == 0), stop=(mc == KC - 1),
                        )
                        nc.tensor.matmul(
                            py[0:qsz, 512:512 + ON2],
                            attnT[mc][:, q0:q1],
                            wp_sb[mc][:, 512:512 + ON2],
                            start=(mc == 0), stop=(mc == KC - 1),
                        )
                    ysb = wk.tile([128, C], _F32, name="ysb", tag="ysb", bufs=3)
                    nc.scalar.copy(ysb[0:qsz, 0:YC], py[0:qsz, 0:YC])
                    nc.sync.dma_start(y_d[s, q0:q1, :], ysb[0:qsz, :])
                    yield

            # ---- software-pipelined driver: proj(i) 1:2 with attn(i-1) ----
            states = {0: emit_load(0)}
            attn_gen = None
            for it in range(niter):
                if it + 1 < niter:
                    states[it + 1] = emit_load(it + 1)
                proj_gen = emit_proj(it, states[it])
                p_done = a_done = False
                while not (p_done and a_done):
                    if not p_done:
                        p_done = next(proj_gen, StopIteration) is StopIteration
                    for _ in range(2):
                        if attn_gen is None:
                            a_done = True
                            break
                        if next(attn_gen, StopIteration) is StopIteration:
                            a_done = True
                            attn_gen = None
                            states.pop(it - 1, None)
                            break
                attn_gen = emit_attn(it, states[it])
            while next(attn_gen, StopIteration) is not StopIteration:
                pass

    _legalize_waits(nc)
    return nc


_NC_CACHE = {}


def _get_module():
    key = (PROJ_DT_NAME, ATT_DT_NAME, PHASES, REPS, tuple(sorted(ABLATE)))
    if key not in _NC_CACHE:
        _NC_CACHE[key] = build_module()
    return _NC_CACHE[key]


def kernel(x, temp_mask, W_qkv, W_proj, b_proj, t_h=None, t_w=None, s_h=None, s_w=None):
    x = np.asarray(x, dtype=np.float32)
    temp_mask = np.asarray(temp_mask, dtype=np.float32)
    B = x.shape[0]
    assert x.shape == (32, N, C), x.shape

    pdt_np = ml_dtypes.bfloat16 if PROJ_DT_NAME == "bfloat16" else np.float32
    wqkvT = np.ascontiguousarray(np.asarray(W_qkv, np.float32).T).astype(pdt_np)
    wpT = np.ascontiguousarray(np.asarray(W_proj, np.float32).T).astype(pdt_np)
    tm = np.ascontiguousarray(temp_mask.reshape(B, 64))

    nc = _get_module()
    per = B // NCORES
    in_maps = [
        {
            "x": np.ascontiguousarray(x[c * per:(c + 1) * per]),
            "tmask": np.ascontiguousarray(tm[c * per:(c + 1) * per]),
            "wqkvT": wqkvT,
            "wpT": wpT,
        }
        for c in range(NCORES)
    ]
    res = run_bass_kernel_spmd(nc, in_maps, core_ids=list(range(NCORES)), trace=TRACE)
    kernel.last_result = res
    y = np.concatenate([res.results[c]["y"] for c in range(NCORES)], axis=0)
    y = y + np.asarray(b_proj, np.float32)[None, None, :]
    return y.astype(np.float32)



# revision 15
# speedup vs baseline: 1.0310x; 1.0310x over previous
"""Trainium2 Bass kernel for masked two-template sparse attention.

Model (per sample, fp32 reference):
    qkv = (x @ W_qkv.T) * mask          mask: temp_mask on first 64 tokens, 1 elsewhere
    q,k,v split into 12 heads x 64
    template tokens (first 128) attend to template tokens only
    search tokens (last 324) attend to all 452 tokens
    out = concat(attn outputs) @ W_proj.T + b_proj

Sharding: data-parallel over batch, 32 samples -> 4 per NeuronCore x 8 cores.
All attention math in "transposed" layout (channels on partitions):
    x^T (PE transpose) -> q^T,k^T = Wqkv^T.T @ x^T ; v natural = x^T.T @ Wv^T
    S^T = k^T.T @ q^T  per key-chunk; the template block IS key-chunk 0 vs
      q 0:128, so chunk 0 streams all 452 queries and the other chunks only
      the 324 search queries. Head pair row-tiled at (0,0)/(64,0) into the
      two banks of one PSUM tile.
    E^T = exp(S^T * scale)             (no max subtraction; |S| <~ 6)
    [attn^T_unnorm ; sums] = [v_h|ones].T @ E^T  (one matmul per head/k-chunk;
                              partitions 0:64 = attn.V, 64:128 = denominator)
    attn^T = attn^T_unnorm * recip(sums)         (partition-shifted DVE ops)
    y = attn^T.T @ Wp^T                (+ bias added on host)

Scheduling: the engine queues are strict FIFO at runtime, so a stalled
attention matmul blocks every later (ready) instruction behind it. To keep
the PE fed, emission is software-pipelined: sample i's projection blocks
are interleaved 1:2 with sample i-1's attention blocks, loads are
prefetched one sample ahead (Pool engine), and PV/norm(p-1) is emitted
after scores(p) so each exp has a full block of slack before its consumer.
"""

import numpy as np
import ml_dtypes

import concourse.bass as bass
import concourse.mybir as mybir
import concourse.tile as tile
from concourse.bass_utils import run_bass_kernel_spmd
from concourse.masks import make_identity

# ---------------- configuration ----------------
PROJ_DT_NAME = "bfloat16"
ATT_DT_NAME = "bfloat16"
TRACE = False        # request NTFF profile on run
PHASES = 99          # kept for test.py compat (unused)
REPS = 1             # timing: repeat the whole computation inside the NEFF
# Timing-ablation switches (results become garbage; ONLY for HW attribution).
# Each entry shrinks the free dim of one instruction class to ~8 while keeping
# instruction count and dependency shape: {"scores","pv","qkproj","vproj",
# "outproj","exp","dve"}
ABLATE = set()
YSB_ON_DVE = False   # drain out-proj PSUM via DVE instead of ACT

NCORES = 8
S = 4                # samples per core
N, C, H, DH = 452, 768, 12, 64
NMT, NS = 128, 324   # template tokens / search tokens
SCALE = DH ** -0.5
TCH = [(0, 128), (128, 256), (256, 384), (384, 452)]  # token/key chunks
KC = 6               # channel chunks of 128
NPAD = 512           # padded token width for x^T storage

_F32 = mybir.dt.float32


def _legalize_waits(nc, max_waits=1):
    """This container's walrus accepts at most one sync-wait per instruction;
    hoist extra waits onto dedicated NOPs in front of the instruction."""
    n_split = 0
    for f in nc.m.functions:
        for bb in f.blocks:
            new_insts = []
            for inst in bb.instructions:
                si = inst.sync_info
                if si is not None and si.on_wait and len(si.on_wait) > max_waits:
                    waits = list(si.on_wait)
                    for i, w in enumerate(waits[:-max_waits]):
                        new_insts.append(
                            mybir.InstNoOp(
                                name=f"{inst.name}-w{i}",
                                sync_info=mybir.SyncInfo(on_wait=[w], on_update=[]),
                                bass_nofuse=True,
                                engine=inst.engine,
                            )
                        )
                    si.on_wait = waits[-max_waits:]
                    n_split += 1
                new_insts.append(inst)
            bb.instructions = new_insts
    return n_split


def build_module():
    pdt = getattr(mybir.dt, PROJ_DT_NAME)
    adt = getattr(mybir.dt, ATT_DT_NAME)

    nc = bass.Bass("TRN2", target_bir_lowering=False, debug=False)
    x_d = nc.dram_tensor("x", [S, N, C], _F32, kind="ExternalInput").ap()
    m_d = nc.dram_tensor("tmask", [S, 64], _F32, kind="ExternalInput").ap()
    wq_d = nc.dram_tensor("wqkvT", [C, 3 * C], pdt, kind="ExternalInput").ap()
    wp_d = nc.dram_tensor("wpT", [C, C], pdt, kind="ExternalInput").ap()
    y_d = nc.dram_tensor("y", [S, N, C], _F32, kind="ExternalOutput").ap()

    Exp = mybir.ActivationFunctionType.Exp

    def sz(cls, full):
        return 8 if cls in ABLATE else full

    with tile.TileContext(nc) as tc:
        with (
            tc.tile_pool(name="const", bufs=1) as cp,
            tc.tile_pool(name="work", bufs=1) as wk,
            # PSUM: 8 banks total.
            #  pps: 2-bank f32 [128,1024] - one tile per score key-chunk,
            #       head 0 in bank0, head 1 in bank1.   2 bufs -> 4 banks
            #  pacc: 1-bank (2KB) tiles - x^T transposes (bf16) and the
            #       qk/v projections (f32).             2 bufs -> 2 banks
            #  ppv: 1-bank f32 [128,512] - PV accumulators and the output
            #       projection.                          2 bufs -> 2 banks
            tc.tile_pool(name="pps", bufs=2, space="PSUM") as pps,
            tc.tile_pool(name="pacc", bufs=2, space="PSUM") as pacc,
            tc.tile_pool(name="ppv", bufs=2, space="PSUM") as ppv,
        ):
            # ---- persistent constants ----
            wq_sb = []
            for i in range(KC):
                w = cp.tile([128, 3 * C], pdt, name=f"wq{i}", tag=f"wq{i}")
                nc.scalar.dma_start(w[:, :], wq_d[i * 128:(i + 1) * 128, :])
                wq_sb.append(w)
            wp_sb = []
            for i in range(KC):
                w = cp.tile([128, C], pdt, name=f"wp{i}", tag=f"wp{i}")
                nc.scalar.dma_start(w[:, :], wp_d[i * 128:(i + 1) * 128, :])
                wp_sb.append(w)
            ident = cp.tile([128, 128], adt, name="ident", tag="ident")
            make_identity(nc, ident)

            niter = REPS * S

            def emit_load(it):
                """DMA + mask + cast for iteration it (runs one sample ahead;
                Pool engine only, so it never blocks the PE pipeline)."""
                s = it % S
                st = {}
                xn = wk.tile([128, 4, C], _F32, name="xn", tag="xn", bufs=2)
                nc.gpsimd.memset(xn[64:128, 3, :], 0.0)
                nc.sync.dma_start(
                    xn[:, 0:3, :],
                    x_d[s, 0:384, :].rearrange("(c p) d -> p c d", p=128),
                )
                nc.sync.dma_start(xn[0:68, 3, :], x_d[s, 384:452, :])
                msk = wk.tile([64, 1], _F32, name="msk", tag="msk", bufs=2)
                nc.sync.dma_start(msk[:, :], m_d[s, :].unsqueeze(1))
                nc.gpsimd.tensor_scalar_mul(xn[0:64, 0, :], xn[0:64, 0, :], msk[0:64, :])
                xnc = wk.tile([128, 4, C], adt, name="xnc", tag="xnc", bufs=2)
                nc.gpsimd.tensor_copy(xnc[:, :, :], xn[:, :, :])
                st["xnc"] = xnc
                return st

            def emit_proj(it, st):
                """x^T transposes + qk/v projections for iteration it.
                Yields after each block (~1us of dense PE work)."""
                xnc = st["xnc"]
                xTb = wk.tile([128, KC, NPAD], pdt, name="xTb", tag="xTb", bufs=2)
                st["xTb"] = xTb
                for ti in range(4):
                    ptr = pacc.tile([128, 1024], adt, name="ptr", tag="acc")
                    for cc in range(KC):
                        nc.tensor.transpose(
                            ptr[:, cc * 128:(cc + 1) * 128],
                            xnc[:, ti, cc * 128:(cc + 1) * 128],
                            ident[:, :],
                        )
                    if "dve" in ABLATE:
                        nc.vector.tensor_copy(
                            xTb[:, 0:1, ti * 128:ti * 128 + 8],
                            ptr.rearrange("p (c k) -> p c k", k=128)[:, 0:1, 0:8],
                        )
                    else:
                        nc.vector.tensor_copy(
                            xTb[:, :, ti * 128:(ti + 1) * 128],
                            ptr.rearrange("p (c k) -> p c k", k=128)[:, 0:KC, :],
                        )
                    yield
                xT = [xTb[:, cc, 0:N] for cc in range(KC)]

                qkT = []
                st["qkT"] = qkT
                QN = sz("qkproj", N)
                DN = sz("dve", N)
                for oc in range(12):
                    pq = pacc.tile([128, 512], _F32, name="pq", tag="acc")
                    for kc in range(KC):
                        nc.tensor.matmul(
                            pq[:, 0:QN],
                            wq_sb[kc][:, oc * 128:(oc + 1) * 128],
                            xTb[:, kc, 0:QN],
                            start=(kc == 0),
                            stop=(kc == KC - 1),
                        )
                    t = wk.tile([128, N], adt, name=f"qkT{oc}", tag=f"qkT{oc}", bufs=2)
                    nc.vector.tensor_copy(t[:, 0:DN], pq[:, 0:DN])
                    qkT.append(t)
                    yield

                vt = []
                st["vt"] = vt
                VN1 = sz("vproj", 512)
                VN2 = sz("vproj", 256)
                for ti, (t0, t1) in enumerate(TCH):
                    tsz = t1 - t0
                    t = wk.tile([128, H * 128], adt, name=f"v{ti}", tag=f"v{ti}", bufs=2)
                    th = t.rearrange("p (h c) -> p h c", c=128)
                    if it < 2:
                        # ones halves persist in the rotating buffers; only
                        # the first pass over each buffer needs the memset
                        nc.gpsimd.memset(th[:, :, DH:128], 1.0)
                    # pva/pvb de-interleaved: each accumulation group runs
                    # back-to-back on one PSUM bank, and the pool ping-pong
                    # gives the copy of one group slack behind the other's
                    # matmuls
                    pva = pacc.tile([128, 512], _F32, name="pva", tag="acc")
                    for kc in range(KC):
                        nc.tensor.matmul(
                            pva[0:tsz, 0:VN1],
                            xTb[:, kc, t0:t1],
                            wq_sb[kc][:, 1536:1536 + VN1],
                            start=(kc == 0),
                            stop=(kc == KC - 1),
                        )
                    if "dve" in ABLATE:
                        nc.vector.tensor_copy(
                            th[0:tsz, 0:1, 0:8],
                            pva[0:tsz, :].rearrange("p (h c) -> p h c", c=DH)[:, 0:1, 0:8],
                        )
                    else:
                        nc.vector.tensor_copy(
                            th[0:tsz, 0:8, 0:DH],
                            pva[0:tsz, :].rearrange("p (h c) -> p h c", c=DH),
                        )
                    yield
                    pvb = pacc.tile([128, 512], _F32, name="pvb", tag="acc")
                    for kc in range(KC):
                        nc.tensor.matmul(
                            pvb[0:tsz, 0:VN2],
                            xTb[:, kc, t0:t1],
                            wq_sb[kc][:, 2048:2048 + VN2],
                            start=(kc == 0),
                            stop=(kc == KC - 1),
                        )
                    if "dve" in ABLATE:
                        nc.vector.tensor_copy(
                            th[0:tsz, 8:9, 0:8],
                            pvb[0:tsz, 0:256].rearrange("p (h c) -> p h c", c=DH)[:, 0:1, 0:8],
                        )
                    else:
                        nc.vector.tensor_copy(
                            th[0:tsz, 8:12, 0:DH],
                            pvb[0:tsz, 0:256].rearrange("p (h c) -> p h c", c=DH),
                        )
                    vt.append(t)
                    yield

            def emit_attn(it, st):
                """Attention + output projection for iteration it. Yields
                after each small block; scores(p) are emitted before
                PV/norm(p-1) so every exp has slack before its consumer."""
                s = it % S
                qkT, vt = st["qkT"], st["vt"]
                es_all = [None] * 6
                st["attnT"] = [None] * 6

                def scores(p):
                    qc, kt = qkT[p], qkT[6 + p]
                    es = [None] * 4
                    es_all[p] = es
                    for kcj in (1, 2, 3, 0):
                        k0, k1 = TCH[kcj]
                        ksz = k1 - k0
                        qlo = 0 if kcj == 0 else NMT
                        qn = N - qlo
                        SQ = sz("scores", qn)
                        EQ = sz("exp", qn)
                        ps = pps.tile([128, 1024], _F32, name="ps", tag="ps")
                        for hh in range(2):
                            b0 = hh * 64
                            nc.tensor.matmul(
                                ps[0:ksz, hh * 512:hh * 512 + SQ],
                                kt[b0:b0 + 64, k0:k1],
                                qc[b0:b0 + 64, qlo:qlo + SQ],
                                start=True, stop=True,
                                tile_position=(b0, 0),
                                skip_group_check=True,
                            )
                        e = wk.tile(
                            [128, 2, qn], adt, name="es", tag=f"es{kcj}", bufs=3
                        )
                        nc.scalar.activation(
                            e[:, :, 0:EQ],
                            ps.rearrange("p (b k) -> p b k", k=512)[:, :, 0:EQ],
                            Exp,
                            scale=SCALE,
                        )
                        es[kcj] = e
                        yield

                def pv_norm(p):
                    es = es_all[p]
                    at = wk.tile(
                        [128, N], pdt, name=f"attnT{p}", tag=f"attnT{p}", bufs=2
                    )
                    st["attnT"][p] = at
                    for hh in range(2):
                        h = 2 * p + hh
                        pvps = ppv.tile([128, 512], _F32, name="pvps", tag="pv")
                        # chunk 0 (template+search, the longest exp) goes
                        # LAST: start=True on chunk 1 clears the bank, chunk
                        # 0 then overwrites cols 0:128 where has_written is
                        # still clear and accumulates on 128:452
                        for j, kcj in enumerate((1, 2, 3, 0)):
                            k0, k1 = TCH[kcj]
                            ksz = k1 - k0
                            qlo = 0 if kcj == 0 else NMT
                            PQ = sz("pv", N - qlo)
                            nc.tensor.matmul(
                                pvps[:, qlo:qlo + PQ],
                                vt[kcj][0:ksz, h * 128:(h + 1) * 128],
                                es[kcj][0:ksz, hh, 0:PQ],
                                start=(j == 0), stop=(j == 3),
                                skip_group_check=True,
                            )
                        DN = sz("dve", N)
                        r = wk.tile([64, N], _F32, name="r", tag="r", bufs=3)
                        nc.vector.reciprocal(r[:, 0:DN], pvps[64:128, 0:DN])
                        nc.vector.tensor_mul(
                            at[hh * 64:(hh + 1) * 64, 0:DN], pvps[0:64, 0:DN],
                            r[:, 0:DN]
                        )
                        yield

                # ready PV(p-1) blocks go BETWEEN the score blocks of
                # pair p, ahead of the ladder's FIFO stall points
                prev = None
                for p in range(6):
                    sg = scores(p)
                    vg = pv_norm(prev) if prev is not None else None
                    for g in (sg, sg, vg, sg, vg, sg):
                        if g is not None and next(g, StopIteration) is not StopIteration:
                            yield
                    prev = p
                yield from pv_norm(5)

                attnT = st["attnT"]
                ON1 = sz("outproj", 512)
                ON2 = sz("outproj", 256)
                YC = sz("exp", 768)
                for (q0, q1) in TCH:
                    qsz = q1 - q0
                    py = pps.tile([128, 1024], _F32, name="py", tag="ps")
                    for mc in range(KC):
                        nc.tensor.matmul(
                            py[0:qsz, 0:ON1],
                            attnT[mc][:, q0:q1],
                            wp_sb[mc][:, 0:ON1],
                            start=(mc == 0), stop=(mc == KC - 1),
                        )
                        nc.tensor.matmul(
                            py[0:qsz, 512:512 + ON2],
                            attnT[mc][:, q0:q1],
                            wp_sb[mc][:, 512:512 + ON2],
                            start=(mc == 0), stop=(mc == KC - 1),
                        )
                    ysb = wk.tile([128, C], _F32, name="ysb", tag="ysb", bufs=3)
                    nc.scalar.copy(ysb[0:qsz, 0:YC], py[0:qsz, 0:YC])
                    nc.sync.dma_start(y_d[s, q0:q1, :], ysb[0:qsz, :])
                    yield

            # ---- software-pipelined driver: proj(i) 1:2 with attn(i-1) ----
            states = {0: emit_load(0)}
            attn_gen = None
            for it in range(niter):
                if it + 1 < niter:
                    states[it + 1] = emit_load(it + 1)
                proj_gen = emit_proj(it, states[it])
                p_done = a_done = False
                while not (p_done and a_done):
                    if not p_done:
                        p_done = next(proj_gen, StopIteration) is StopIteration
                    for _ in range(2):
                        if attn_gen is None:
                            a_done = True
                            break
                        if next(attn_gen, StopIteration) is StopIteration:
                            a_done = True
                            attn_gen = None
                            states.pop(it - 1, None)
                            break
                attn_gen = emit_attn(it, states[it])
            while next(attn_gen, StopIteration) is not StopIteration:
                pass

    _legalize_waits(nc)
    return nc


_NC_CACHE = {}


def _get_module():
    key = (PROJ_DT_NAME, ATT_DT_NAME, PHASES, REPS, tuple(sorted(ABLATE)))
    if key not in _NC_CACHE:
        _NC_CACHE[key] = build_module()
    return _NC_CACHE[key]


def kernel(x, temp_mask, W_qkv, W_proj, b_proj, t_h=None, t_w=None, s_h=None, s_w=None):
    x = np.asarray(x, dtype=np.float32)
    temp_mask = np.asarray(temp_mask, dtype=np.float32)
    B = x.shape[0]
    assert x.shape == (32, N, C), x.shape

    pdt_np = ml_dtypes.bfloat16 if PROJ_DT_NAME == "bfloat16" else np.float32
    wqkvT = np.ascontiguousarray(np.asarray(W_qkv, np.float32).T).astype(pdt_np)
    wpT = np.ascontiguousarray(np.asarray(W_proj, np.float32).T).astype(pdt_np)
    tm = np.ascontiguousarray(temp_mask.reshape(B, 64))

    nc = _get_module()
    per = B // NCORES
    in_maps = [
        {
            "x": np.ascontiguousarray(x[c * per:(c + 1) * per]),
            "tmask": np.ascontiguousarray(tm[c * per:(c + 1) * per]),
            "wqkvT": wqkvT,
            "wpT": wpT,
        }
        for c in range(NCORES)
    ]
    res = run_bass_kernel_spmd(nc, in_maps, core_ids=list(range(NCORES)), trace=TRACE)
    kernel.last_result = res
    y = np.concatenate([res.results[c]["y"] for c in range(NCORES)], axis=0)
    y = y + np.asarray(b_proj, np.float32)[None, None, :]
    return y.astype(np.float32)



# revision 20
# speedup vs baseline: 1.2112x; 1.1748x over previous
"""Trainium2 Bass kernel for masked two-template sparse attention.

Model (per sample, fp32 reference):
    qkv = (x @ W_qkv.T) * mask          mask: temp_mask on first 64 tokens, 1 elsewhere
    q,k,v split into 12 heads x 64
    template tokens (first 128) attend to template tokens only
    search tokens (last 324) attend to all 452 tokens
    out = concat(attn outputs) @ W_proj.T + b_proj

Sharding: data-parallel over batch, 32 samples -> 4 per NeuronCore x 8 cores.
All attention math in "transposed" layout (channels on partitions):
    x^T (PE transpose) -> q^T,k^T = Wqkv^T.T @ x^T ; v natural = x^T.T @ Wv^T
    S^T = k^T.T @ q^T  per key-chunk; the template block IS key-chunk 0 vs
      q 0:128, so chunk 0 streams all 452 queries and the other chunks only
      the 324 search queries. Head pair row-tiled at (0,0)/(64,0) into the
      two banks of one PSUM tile.
    E^T = exp(S^T * scale)             (no max subtraction; |S| <~ 6)
    [attn^T_unnorm ; sums] = [v_h|ones].T @ E^T  (one matmul per head/k-chunk;
                              partitions 0:64 = attn.V, 64:128 = denominator)
    attn^T = attn^T_unnorm * recip(sums)         (partition-shifted DVE ops)
    y = attn^T.T @ Wp^T                (+ bias added on host)

Scheduling: the engine queues are strict FIFO at runtime, so a stalled
attention matmul blocks every later (ready) instruction behind it. To keep
the PE fed, emission is software-pipelined: sample i's projection blocks
are interleaved 1:2 with sample i-1's attention blocks, loads are
prefetched one sample ahead (Pool engine), and PV/norm(p-1) is emitted
after scores(p) so each exp has a full block of slack before its consumer.
"""

import numpy as np
import ml_dtypes

import concourse.bass as bass
import concourse.mybir as mybir
import concourse.tile as tile
from concourse.bass_utils import run_bass_kernel_spmd
from concourse.masks import make_identity

# ---------------- configuration ----------------
PROJ_DT_NAME = "bfloat16"
ATT_DT_NAME = "bfloat16"
TRACE = False        # request NTFF profile on run
PHASES = 99          # kept for test.py compat (unused)
REPS = 1             # timing: repeat the whole computation inside the NEFF
# Timing-ablation switches (results become garbage; ONLY for HW attribution).
# Each entry shrinks the free dim of one instruction class to ~8 while keeping
# instruction count and dependency shape: {"scores","pv","qkproj","vproj",
# "outproj","exp","dve"}
ABLATE = set()
YSB_ON_DVE = False   # drain out-proj PSUM via DVE instead of ACT
# De-interleaved pva/pvb accumulation groups measured ~10-27us/rep faster on
# HW than the interleaved order (HAM clock-gate oscillation is sensitive to
# PSUM-bank group cycling; contemporaneous A/B, R=33 and R=49 slopes).
INTERLEAVE_PV = False

NCORES = 8
S = 4                # samples per core
N, C, H, DH = 452, 768, 12, 64
NMT, NS = 128, 324   # template tokens / search tokens
SCALE = DH ** -0.5
TCH = [(0, 128), (128, 256), (256, 384), (384, 452)]  # token/key chunks
KC = 6               # channel chunks of 128
NPAD = 512           # padded token width for x^T storage

_F32 = mybir.dt.float32


def _legalize_waits(nc, max_waits=1):
    """This container's walrus accepts at most one sync-wait per instruction;
    hoist extra waits onto dedicated NOPs in front of the instruction."""
    n_split = 0
    for f in nc.m.functions:
        for bb in f.blocks:
            new_insts = []
            for inst in bb.instructions:
                si = inst.sync_info
                if si is not None and si.on_wait and len(si.on_wait) > max_waits:
                    waits = list(si.on_wait)
                    for i, w in enumerate(waits[:-max_waits]):
                        new_insts.append(
                            mybir.InstNoOp(
                                name=f"{inst.name}-w{i}",
                                sync_info=mybir.SyncInfo(on_wait=[w], on_update=[]),
                                bass_nofuse=True,
                                engine=inst.engine,
                            )
                        )
                    si.on_wait = waits[-max_waits:]
                    n_split += 1
                new_insts.append(inst)
            bb.instructions = new_insts
    return n_split


def build_module():
    pdt = getattr(mybir.dt, PROJ_DT_NAME)
    adt = getattr(mybir.dt, ATT_DT_NAME)

    nc = bass.Bass("TRN2", target_bir_lowering=False, debug=False)
    x_d = nc.dram_tensor("x", [S, N, C], _F32, kind="ExternalInput").ap()
    m_d = nc.dram_tensor("tmask", [S, 64], _F32, kind="ExternalInput").ap()
    wq_d = nc.dram_tensor("wqkvT", [C, 3 * C], pdt, kind="ExternalInput").ap()
    wp_d = nc.dram_tensor("wpT", [C, C], pdt, kind="ExternalInput").ap()
    y_d = nc.dram_tensor("y", [S, N, C], _F32, kind="ExternalOutput").ap()

    Exp = mybir.ActivationFunctionType.Exp

    def sz(cls, full):
        return 8 if cls in ABLATE else full

    with tile.TileContext(nc) as tc:
        with (
            tc.tile_pool(name="const", bufs=1) as cp,
            tc.tile_pool(name="work", bufs=1) as wk,
            # PSUM: 8 banks total.
            #  pps: 2-bank f32 [128,1024] - one tile per score key-chunk,
            #       head 0 in bank0, head 1 in bank1.   2 bufs -> 4 banks
            #  pacc: 1-bank (2KB) tiles - x^T transposes (bf16) and the
            #       qk/v projections (f32).             2 bufs -> 2 banks
            #  ppv: 1-bank f32 [128,512] - PV accumulators and the output
            #       projection.                          2 bufs -> 2 banks
            tc.tile_pool(name="pps", bufs=2, space="PSUM") as pps,
            tc.tile_pool(name="pacc", bufs=2, space="PSUM") as pacc,
            tc.tile_pool(name="ppv", bufs=2, space="PSUM") as ppv,
        ):
            # ---- persistent constants ----
            wq_sb = []
            for i in range(KC):
                w = cp.tile([128, 3 * C], pdt, name=f"wq{i}", tag=f"wq{i}")
                nc.scalar.dma_start(w[:, :], wq_d[i * 128:(i + 1) * 128, :])
                wq_sb.append(w)
            wp_sb = []
            for i in range(KC):
                w = cp.tile([128, C], pdt, name=f"wp{i}", tag=f"wp{i}")
                nc.scalar.dma_start(w[:, :], wp_d[i * 128:(i + 1) * 128, :])
                wp_sb.append(w)
            ident = cp.tile([128, 128], adt, name="ident", tag="ident")
            make_identity(nc, ident)

            niter = REPS * S

            def emit_load(it):
                """DMA + mask + cast for iteration it (runs one sample ahead;
                Pool engine only, so it never blocks the PE pipeline)."""
                s = it % S
                st = {}
                xn = wk.tile([128, 4, C], _F32, name="xn", tag="xn", bufs=2)
                nc.gpsimd.memset(xn[64:128, 3, :], 0.0)
                nc.sync.dma_start(
                    xn[:, 0:3, :],
                    x_d[s, 0:384, :].rearrange("(c p) d -> p c d", p=128),
                )
                nc.sync.dma_start(xn[0:68, 3, :], x_d[s, 384:452, :])
                msk = wk.tile([64, 1], _F32, name="msk", tag="msk", bufs=2)
                nc.sync.dma_start(msk[:, :], m_d[s, :].unsqueeze(1))
                nc.gpsimd.tensor_scalar_mul(xn[0:64, 0, :], xn[0:64, 0, :], msk[0:64, :])
                xnc = wk.tile([128, 4, C], adt, name="xnc", tag="xnc", bufs=2)
                nc.gpsimd.tensor_copy(xnc[:, :, :], xn[:, :, :])
                st["xnc"] = xnc
                return st

            def emit_proj(it, st):
                """x^T transposes + qk/v projections for iteration it.
                Yields after each block (~1us of dense PE work)."""
                xnc = st["xnc"]
                xTb = wk.tile([128, KC, NPAD], pdt, name="xTb", tag="xTb", bufs=2)
                st["xTb"] = xTb
                for ti in range(4):
                    ptr = pacc.tile([128, 1024], adt, name="ptr", tag="acc")
                    for cc in range(KC):
                        nc.tensor.transpose(
                            ptr[:, cc * 128:(cc + 1) * 128],
                            xnc[:, ti, cc * 128:(cc + 1) * 128],
                            ident[:, :],
                        )
                    if "dve" in ABLATE:
                        nc.vector.tensor_copy(
                            xTb[:, 0:1, ti * 128:ti * 128 + 8],
                            ptr.rearrange("p (c k) -> p c k", k=128)[:, 0:1, 0:8],
                        )
                    else:
                        nc.vector.tensor_copy(
                            xTb[:, :, ti * 128:(ti + 1) * 128],
                            ptr.rearrange("p (c k) -> p c k", k=128)[:, 0:KC, :],
                        )
                    yield
                xT = [xTb[:, cc, 0:N] for cc in range(KC)]

                qkT = []
                st["qkT"] = qkT
                QN = sz("qkproj", N)
                DN = sz("dve", N)
                for oc in range(12):
                    pq = pacc.tile([128, 512], _F32, name="pq", tag="acc")
                    for kc in range(KC):
                        nc.tensor.matmul(
                            pq[:, 0:QN],
                            wq_sb[kc][:, oc * 128:(oc + 1) * 128],
                            xTb[:, kc, 0:QN],
                            start=(kc == 0),
                            stop=(kc == KC - 1),
                        )
                    t = wk.tile([128, N], adt, name=f"qkT{oc}", tag=f"qkT{oc}", bufs=2)
                    nc.vector.tensor_copy(t[:, 0:DN], pq[:, 0:DN])
                    qkT.append(t)
                    yield

                vt = []
                st["vt"] = vt
                VN1 = sz("vproj", 512)
                VN2 = sz("vproj", 256)
                for ti, (t0, t1) in enumerate(TCH):
                    tsz = t1 - t0
                    t = wk.tile([128, H * 128], adt, name=f"v{ti}", tag=f"v{ti}", bufs=2)
                    th = t.rearrange("p (h c) -> p h c", c=128)
                    if it < 2:
                        # ones halves persist in the rotating buffers; only
                        # the first pass over each buffer needs the memset
                        nc.gpsimd.memset(th[:, :, DH:128], 1.0)
                    pva = pacc.tile([128, 512], _F32, name="pva", tag="acc")
                    pvb = pacc.tile([128, 512], _F32, name="pvb", tag="acc")
                    if INTERLEAVE_PV:
                        for kc in range(KC):
                            nc.tensor.matmul(
                                pva[0:tsz, 0:VN1],
                                xTb[:, kc, t0:t1],
                                wq_sb[kc][:, 1536:1536 + VN1],
                                start=(kc == 0),
                                stop=(kc == KC - 1),
                            )
                            nc.tensor.matmul(
                                pvb[0:tsz, 0:VN2],
                                xTb[:, kc, t0:t1],
                                wq_sb[kc][:, 2048:2048 + VN2],
                                start=(kc == 0),
                                stop=(kc == KC - 1),
                            )
                    else:
                        for kc in range(KC):
                            nc.tensor.matmul(
                                pva[0:tsz, 0:VN1],
                                xTb[:, kc, t0:t1],
                                wq_sb[kc][:, 1536:1536 + VN1],
                                start=(kc == 0),
                                stop=(kc == KC - 1),
                            )
                        for kc in range(KC):
                            nc.tensor.matmul(
                                pvb[0:tsz, 0:VN2],
                                xTb[:, kc, t0:t1],
                                wq_sb[kc][:, 2048:2048 + VN2],
                                start=(kc == 0),
                                stop=(kc == KC - 1),
                            )
                    if "dve" in ABLATE:
                        nc.vector.tensor_copy(
                            th[0:tsz, 0:1, 0:8],
                            pva[0:tsz, :].rearrange("p (h c) -> p h c", c=DH)[:, 0:1, 0:8],
                        )
                        nc.vector.tensor_copy(
                            th[0:tsz, 8:9, 0:8],
                            pvb[0:tsz, 0:256].rearrange("p (h c) -> p h c", c=DH)[:, 0:1, 0:8],
                        )
                    else:
                        nc.vector.tensor_copy(
                            th[0:tsz, 0:8, 0:DH],
                            pva[0:tsz, :].rearrange("p (h c) -> p h c", c=DH),
                        )
                        nc.vector.tensor_copy(
                            th[0:tsz, 8:12, 0:DH],
                            pvb[0:tsz, 0:256].rearrange("p (h c) -> p h c", c=DH),
                        )
                    vt.append(t)
                    yield

            def emit_attn(it, st):
                """Attention + output projection for iteration it. Yields
                after each small block; scores(p) are emitted before
                PV/norm(p-1) so every exp has slack before its consumer."""
                s = it % S
                qkT, vt = st["qkT"], st["vt"]
                es_all = [None] * 6
                st["attnT"] = [None] * 6

                def scores(p):
                    qc, kt = qkT[p], qkT[6 + p]
                    es = [None] * 4
                    es_all[p] = es
                    for kcj in (1, 2, 3, 0):
                        k0, k1 = TCH[kcj]
                        ksz = k1 - k0
                        qlo = 0 if kcj == 0 else NMT
                        qn = N - qlo
                        SQ = sz("scores", qn)
                        EQ = sz("exp", qn)
                        ps = pps.tile([128, 1024], _F32, name="ps", tag="ps")
                        for hh in range(2):
                            b0 = hh * 64
                            nc.tensor.matmul(
                                ps[0:ksz, hh * 512:hh * 512 + SQ],
                                kt[b0:b0 + 64, k0:k1],
                                qc[b0:b0 + 64, qlo:qlo + SQ],
                                start=True, stop=True,
                                tile_position=(b0, 0),
                                skip_group_check=True,
                            )
                        e = wk.tile(
                            [128, 2, qn], adt, name="es", tag=f"es{kcj}", bufs=3
                        )
                        nc.scalar.activation(
                            e[:, :, 0:EQ],
                            ps.rearrange("p (b k) -> p b k", k=512)[:, :, 0:EQ],
                            Exp,
                            scale=SCALE,
                        )
                        es[kcj] = e
                        yield

                def pv_norm(p):
                    es = es_all[p]
                    at = wk.tile(
                        [128, N], pdt, name=f"attnT{p}", tag=f"attnT{p}", bufs=2
                    )
                    st["attnT"][p] = at
                    for hh in range(2):
                        h = 2 * p + hh
                        pvps = ppv.tile([128, 512], _F32, name="pvps", tag="pv")
                        # chunk 0 (template+search, the longest exp) goes
                        # LAST: start=True on chunk 1 clears the bank, chunk
                        # 0 then overwrites cols 0:128 where has_written is
                        # still clear and accumulates on 128:452
                        for j, kcj in enumerate((1, 2, 3, 0)):
                            k0, k1 = TCH[kcj]
                            ksz = k1 - k0
                            qlo = 0 if kcj == 0 else NMT
                            PQ = sz("pv", N - qlo)
                            nc.tensor.matmul(
                                pvps[:, qlo:qlo + PQ],
                                vt[kcj][0:ksz, h * 128:(h + 1) * 128],
                                es[kcj][0:ksz, hh, 0:PQ],
                                start=(j == 0), stop=(j == 3),
                                skip_group_check=True,
                            )
                        DN = sz("dve", N)
                        r = wk.tile([64, N], _F32, name="r", tag="r", bufs=3)
                        nc.vector.reciprocal(r[:, 0:DN], pvps[64:128, 0:DN])
                        nc.vector.tensor_mul(
                            at[hh * 64:(hh + 1) * 64, 0:DN], pvps[0:64, 0:DN],
                            r[:, 0:DN]
                        )
                        yield

                # ready PV(p-1) blocks go BETWEEN the score blocks of
                # pair p, ahead of the ladder's FIFO stall points
                prev = None
                for p in range(6):
                    sg = scores(p)
                    vg = pv_norm(prev) if prev is not None else None
                    for g in (sg, sg, vg, sg, vg, sg):
                        if g is not None and next(g, StopIteration) is not StopIteration:
                            yield
                    prev = p
                yield from pv_norm(5)

                attnT = st["attnT"]
                ON1 = sz("outproj", 512)
                ON2 = sz("outproj", 256)
                YC = sz("exp", 768)
                for (q0, q1) in TCH:
                    qsz = q1 - q0
                    py = pps.tile([128, 1024], _F32, name="py", tag="ps")
                    for mc in range(KC):
                        nc.tensor.matmul(
                            py[0:qsz, 0:ON1],
                            attnT[mc][:, q0:q1],
                            wp_sb[mc][:, 0:ON1],
                            start=(mc == 0), stop=(mc == KC - 1),
                        )
                        nc.tensor.matmul(
                            py[0:qsz, 512:512 + ON2],
                            attnT[mc][:, q0:q1],
                            wp_sb[mc][:, 512:512 + ON2],
                            start=(mc == 0), stop=(mc == KC - 1),
                        )
                    ysb = wk.tile([128, C], _F32, name="ysb", tag="ysb", bufs=3)
                    if YSB_ON_DVE:
                        nc.vector.tensor_copy(ysb[0:qsz, 0:YC], py[0:qsz, 0:YC])
                    else:
                        nc.scalar.copy(ysb[0:qsz, 0:YC], py[0:qsz, 0:YC])
                    nc.sync.dma_start(y_d[s, q0:q1, :], ysb[0:qsz, :])
                    yield

            # ---- software-pipelined driver: proj(i) 1:2 with attn(i-1) ----
            states = {0: emit_load(0)}
            attn_gen = None
            for it in range(niter):
                if it + 1 < niter:
                    states[it + 1] = emit_load(it + 1)
                proj_gen = emit_proj(it, states[it])
                p_done = a_done = False
                while not (p_done and a_done):
                    if not p_done:
                        p_done = next(proj_gen, StopIteration) is StopIteration
                    for _ in range(2):
                        if attn_gen is None:
                            a_done = True
                            break
                        if next(attn_gen, StopIteration) is StopIteration:
                            a_done = True
                            attn_gen = None
                            states.pop(it - 1, None)
                            break
                attn_gen = emit_attn(it, states[it])
            while next(attn_gen, StopIteration) is not StopIteration:
                pass

    _legalize_waits(nc)
    return nc


_NC_CACHE = {}


def _get_module():
    key = (PROJ_DT_NAME, ATT_DT_NAME, PHASES, REPS, tuple(sorted(ABLATE)),
           YSB_ON_DVE, INTERLEAVE_PV)
    if key not in _NC_CACHE:
        _NC_CACHE[key] = build_module()
    return _NC_CACHE[key]


def kernel(x, temp_mask, W_qkv, W_proj, b_proj, t_h=None, t_w=None, s_h=None, s_w=None):
    x = np.asarray(x, dtype=np.float32)
    temp_mask = np.asarray(temp_mask, dtype=np.float32)
    B = x.shape[0]
    assert x.shape == (32, N, C), x.shape

    pdt_np = ml_dtypes.bfloat16 if PROJ_DT_NAME == "bfloat16" else np.float32
    wqkvT = np.ascontiguousarray(np.asarray(W_qkv, np.float32).T).astype(pdt_np)
    wpT = np.ascontiguousarray(np.asarray(W_proj, np.float32).T).astype(pdt_np)
    tm = np.ascontiguousarray(temp_mask.reshape(B, 64))

    nc = _get_module()
    per = B // NCORES
    in_maps = [
        {
            "x": np.ascontiguousarray(x[c * per:(c + 1) * per]),
            "tmask": np.ascontiguousarray(tm[c * per:(c + 1) * per]),
            "wqkvT": wqkvT,
            "wpT": wpT,
        }
        for c in range(NCORES)
    ]
    res = run_bass_kernel_spmd(nc, in_maps, core_ids=list(range(NCORES)), trace=TRACE)
    kernel.last_result = res
    y = np.concatenate([res.results[c]["y"] for c in range(NCORES)], axis=0)
    y = y + np.asarray(b_proj, np.float32)[None, None, :]
    return y.astype(np.float32)

